# revision 1
# baseline (speedup 1.0000x reference)
"""Segment-mean + linear head kernel for TRN2 (8 NeuronCores, data parallel).

Reference computation (per batch row r):
    seg-mean of x[r] over tokens sharing word_id, gathered back per token,
    then linear head W,b:  logits[r,s,:] = mean_{s': wid[s']=wid[s]} x[r,s'] @ W.T + b

Key identity: the mean and the linear head commute, so
    logits[r,s,:] = Z[wid[s],:]  with  Z[g,:] = (sum_{s in g} y[s,:]) / max(cnt_g,1) + b,
    y = x @ W.T   ([S,15] -- tiny channel dim).
The segment scatter/gather is done with 0/1 indicator matmuls on the tensor
engine; indicators are generated on-chip with iota + is_equal compares.
Word ids are sorted per row, so each 128-wide segment chunk only touches a
few contiguous 128-token tiles; that schedule is computed on the host from
the actual ids (union across cores so the SPMD program is identical).
"""

import sys
from contextlib import ExitStack

import numpy as np

for _p in ("/opt/trn_rl_repo",):
    if _p not in sys.path:
        sys.path.insert(0, _p)

import concourse.bass as bass
import concourse.bacc as bacc
import concourse.tile as tile
from concourse import mybir
from concourse.bass_utils import run_bass_kernel_spmd

B, S, H, C = 16, 2048, 1024, 15
NW = 800
NCORES = 8
RPC = B // NCORES          # rows per core
T = S // 128               # 128-token tiles per row
NK = H // 128              # 128-wide h chunks
NCHUNK = (NW + 127) // 128 # 128-wide segment chunks

F32 = mybir.dt.float32
F32R = mybir.dt.float32r
BF16 = mybir.dt.bfloat16
I32 = mybir.dt.int32
EQ = mybir.AluOpType.is_equal
MULT = mybir.AluOpType.mult


def _schedule(word_ids):
    """chunks_t[lr][t]: sorted segment-chunk ids present in tile t of local row
    lr on ANY core; windows[lr][j]: sorted tiles where chunk j is active."""
    cid = (np.asarray(word_ids).astype(np.int64) // 128).reshape(B, T, 128)
    chunks_t = [[set() for _ in range(T)] for _ in range(RPC)]
    for core in range(NCORES):
        for lr in range(RPC):
            g = core * RPC + lr
            for t in range(T):
                for j in np.unique(cid[g, t]):
                    chunks_t[lr][t].add(int(j))
    chunks_t = [[sorted(s) for s in row] for row in chunks_t]
    windows = [
        [[t for t in range(T) if j in chunks_t[lr][t]] for j in range(NCHUNK)]
        for lr in range(RPC)
    ]
    return chunks_t, windows


def _build(chunks_t, windows):
    nc = bacc.Bacc("TRN2", target_bir_lowering=False, debug=False)
    x_d = nc.declare_dram_parameter("x", [RPC, S, H], BF16, isOutput=False)
    widr_d = nc.declare_dram_parameter("widr", [RPC, S], F32R, isOutput=False)
    widc_d = nc.declare_dram_parameter("widc", [RPC, 128, T], F32, isOutput=False)
    wt_d = nc.declare_dram_parameter("wt", [NK, 128, C], BF16, isOutput=False)
    b_d = nc.declare_dram_parameter("bias", [1, 16], F32R, isOutput=False)
    out_d = nc.declare_dram_parameter("out", [RPC, 128, T * C], F32, isOutput=True)

    with tile.TileContext(nc) as tc, ExitStack() as ctx:
        consts = ctx.enter_context(tc.tile_pool(name="consts", bufs=1))
        widp = ctx.enter_context(tc.tile_pool(name="widp", bufs=2))
        xpool = ctx.enter_context(tc.tile_pool(name="xpool", bufs=3))
        xtpool = ctx.enter_context(tc.tile_pool(name="xtpool", bufs=2))
        ytsb = ctx.enter_context(tc.tile_pool(name="ytsb", bufs=2))
        y1p = ctx.enter_context(tc.tile_pool(name="y1p", bufs=4))
        apool = ctx.enter_context(tc.tile_pool(name="apool", bufs=4))
        zpool = ctx.enter_context(tc.tile_pool(name="zpool", bufs=2))
        scp = ctx.enter_context(tc.tile_pool(name="scp", bufs=4))
        opool = ctx.enter_context(tc.tile_pool(name="opool", bufs=2))
        tpps = ctx.enter_context(tc.tile_pool(name="tpps", bufs=3, space="PSUM"))
        ypps = ctx.enter_context(tc.tile_pool(name="ypps", bufs=2, space="PSUM"))
        smps = ctx.enter_context(tc.tile_pool(name="smps", bufs=2, space="PSUM"))

        # --- constants ---
        iotag = consts.tile([128, NCHUNK, 128], F32, tag="iotag")
        nc.gpsimd.iota(iotag[:], [[128, NCHUNK], [1, 128]], channel_multiplier=0,
                       allow_small_or_imprecise_dtypes=True)
        pidx = consts.tile([128, NCHUNK], F32, tag="pidx")
        nc.gpsimd.iota(pidx[:], [[128, NCHUNK]], channel_multiplier=1,
                       allow_small_or_imprecise_dtypes=True)
        i0 = consts.tile([128, 128], F32, tag="i0")
        nc.gpsimd.iota(i0[:], [[1, 128]], channel_multiplier=0,
                       allow_small_or_imprecise_dtypes=True)
        p0 = consts.tile([128, 1], F32, tag="p0")
        nc.gpsimd.iota(p0[:], [[0, 1]], channel_multiplier=1,
                       allow_small_or_imprecise_dtypes=True)
        ident = consts.tile([128, 128], F32, tag="ident")
        nc.vector.tensor_scalar(ident[:], i0[:], p0[:], None, op0=EQ)
        ident_bf = consts.tile([128, 128], BF16, tag="identbf")
        nc.vector.tensor_scalar(ident_bf[:], i0[:], p0[:], None, op0=EQ)
        wt_sb = consts.tile([128, NK, C], BF16, tag="wt")
        nc.sync.dma_start(wt_sb[:], wt_d.rearrange("k h c -> h k c"))
        b_sb = consts.tile([1, 16], F32R, tag="bias")
        nc.sync.dma_start(b_sb[:], b_d[:])
        ones_col = consts.tile([1, 128], F32R, tag="ones")
        nc.vector.memset(ones_col[:].bitcast(F32), 1.0)
        b_bc = consts.tile([128, 16], BF16, tag="bbc")
        bb_ps = smps.tile([128, 16], F32, tag="sm")
        nc.tensor.matmul(bb_ps[:], ones_col[:], b_sb[:], start=True, stop=True)
        nc.any.tensor_copy(b_bc[:], bb_ps[:])

        for r in range(RPC):
            ct = chunks_t[r]
            win = windows[r]
            present = [j for j in range(NCHUNK) if win[j]]

            widr_sb = widp.tile([1, S], F32R, tag="widr")
            nc.sync.dma_start(widr_sb[:], widr_d[r : r + 1, :])
            widc_sb = widp.tile([128, T], F32, tag="widc")
            nc.sync.dma_start(widc_sb[:], widc_d[r])
            xr = x_d[r].rearrange("(t p) h -> p t h", p=128)
            wid_bc = widp.tile([128, S], F32, tag="widbc")
            for q in range(S // 512):
                wb_ps = tpps.tile([128, 512], F32, tag="tp")
                nc.tensor.matmul(
                    wb_ps[:],
                    ones_col[:],
                    widr_sb[0:1, 512 * q : 512 * q + 512],
                    start=True,
                    stop=True,
                )
                nc.any.tensor_copy(wid_bc[:, 512 * q : 512 * q + 512], wb_ps[:])

            sums_sb = zpool.tile([128, NCHUNK, 16], F32, tag="sums")
            nc.vector.memset(sums_sb[:], 0.0)
            # --- pass 1: y = x@W.T per token, scatter into segment sums ---
            for g4 in range(T // 4):
                x_sb = xpool.tile([128, 4, H], BF16)
                nc.sync.dma_start(x_sb[:], xr[:, 4 * g4 : 4 * g4 + 4, :])
                xt_sb = xtpool.tile([128, NK, 512], BF16)
                for ti in range(4):
                    for half in range(2):
                        tp = tpps.tile([128, 512], BF16, tag="tp")
                        for kk in range(4):
                            k = 4 * half + kk
                            nc.tensor.transpose(
                                tp[:, 128 * kk : 128 * kk + 128],
                                x_sb[:, ti, 128 * k : 128 * k + 128],
                                ident_bf[:],
                            )
                        nc.any.tensor_copy(
                            xt_sb[:, 4 * half : 4 * half + 4, 128 * ti : 128 * ti + 128],
                            tp[:].rearrange("p (k s) -> p k s", k=4),
                        )
                yp = ypps.tile([C, 512], F32)
                for k in range(NK):
                    nc.tensor.matmul(
                        yp[:],
                        wt_sb[:, k, :],
                        xt_sb[:, k, :],
                        start=(k == 0),
                        stop=(k == NK - 1),
                    )
                yt = ytsb.tile([C, 512], BF16)
                nc.any.tensor_copy(yt[:], yp[:])
                for ti in range(4):
                    t = 4 * g4 + ti
                    ytp = smps.tile([128, 16], BF16, tag="sm")
                    nc.tensor.transpose(
                        ytp[:, 0:C],
                        yt[:, 128 * ti : 128 * ti + 128],
                        ident_bf[:C, :C],
                    )
                    y1 = y1p.tile([128, 16], BF16)
                    nc.any.tensor_copy(y1[:, 0:C], ytp[:, 0:C])
                    nc.vector.memset(y1[:, C : C + 1], 1.0)
                    for j in ct[t]:
                        a = apool.tile([128, 128], BF16, tag="a")
                        nc.vector.tensor_scalar(
                            a[:], iotag[:, j, :], widc_sb[:, t : t + 1], None, op0=EQ
                        )
                        part = smps.tile([128, 16], F32, tag="sm")
                        nc.tensor.matmul(
                            part[:],
                            a[:],
                            y1[:],
                            start=True,
                            stop=True,
                        )
                        nc.vector.tensor_add(
                            sums_sb[:, j, :], sums_sb[:, j, :], part[:]
                        )

            # --- Z: means + bias per segment chunk ---
            z_sb = zpool.tile([128, NCHUNK, 16], BF16, tag="z")
            for j in present:
                cm = scp.tile([128, 1], F32, tag="cm")
                nc.vector.tensor_scalar_max(cm[:], sums_sb[:, j, C : C + 1], 1.0)
                rc = scp.tile([128, 1], F32, tag="rc")
                nc.vector.reciprocal(rc[:], cm[:])
                nc.vector.tensor_scalar(
                    z_sb[:, j, :], sums_sb[:, j, :], rc[:], None, op0=MULT
                )
                nc.vector.tensor_add(z_sb[:, j, :], z_sb[:, j, :], b_bc[:])

            # --- pass 2: gather Z back to tokens ---
            orow = opool.tile([128, T * C], F32)
            for t in range(T):
                ops_ = smps.tile([128, 16], F32, tag="sm")
                cl = ct[t]
                for idx, j in enumerate(cl):
                    at = apool.tile([128, 128], BF16, tag="a")
                    nc.vector.tensor_scalar(
                        at[:],
                        wid_bc[:, 128 * t : 128 * t + 128],
                        pidx[:, j : j + 1],
                        None,
                        op0=EQ,
                    )
                    nc.tensor.matmul(
                        ops_[:],
                        at[:],
                        z_sb[:, j, :],
                        start=(idx == 0),
                        stop=(idx == len(cl) - 1),
                    )
                nc.any.tensor_copy(orow[:, C * t : C * t + C], ops_[:, 0:C])
            nc.sync.dma_start(out_d[r], orow[:])

    nc.compile()
    return nc


def _prep_host(x, word_ids, W, b):
    import ml_dtypes
    wid32 = np.ascontiguousarray(np.asarray(word_ids).astype(np.int64))
    widf = wid32.astype(np.float32)
    widc = np.ascontiguousarray(
        widf.reshape(B, T, 128).transpose(0, 2, 1)
    )  # [B,128,T]
    wtk = np.ascontiguousarray(
        np.asarray(W, dtype=np.float32).T.reshape(NK, 128, C)
    ).astype(ml_dtypes.bfloat16)
    bp = np.zeros((1, 16), dtype=np.float32)
    bp[0, :C] = np.asarray(b, dtype=np.float32)
    return wid32, widf, widc, wtk, bp


def _run(x, word_ids, W, b, **spmd_kwargs):
    import ml_dtypes
    x = np.ascontiguousarray(np.asarray(x, dtype=np.float32)).astype(ml_dtypes.bfloat16)
    wid32, widf, widc, wtk, bp = _prep_host(x, word_ids, W, b)
    chunks_t, windows = _schedule(wid32)
    nc = _build(chunks_t, windows)

    in_maps = []
    for core in range(NCORES):
        r0 = core * RPC
        in_maps.append(
            {
                "x": x[r0 : r0 + RPC],
                "widr": widf[r0 : r0 + RPC],
                "widc": widc[r0 : r0 + RPC],
                "wt": wtk,
                "bias": bp,
            }
        )
    res = run_bass_kernel_spmd(nc, in_maps, list(range(NCORES)), **spmd_kwargs)
    outs = []
    for core in range(NCORES):
        o = res.results[core]["out"]  # [RPC, 128, T*C]
        o = o.reshape(RPC, 128, T, C).transpose(0, 2, 1, 3).reshape(RPC, S, C)
        outs.append(o)
    full = np.ascontiguousarray(np.concatenate(outs, axis=0).astype(np.float32))
    return full, res


def kernel(x, word_ids, W, b):
    return _run(x, word_ids, W, b)[0]


if __name__ == "__main__":
    rng = np.random.default_rng(0)
    x = rng.standard_normal((B, S, H), dtype=np.float32)
    wid = np.sort(rng.integers(0, NW, (B, S)), axis=-1)
    W = rng.standard_normal((C, H), dtype=np.float32) / np.sqrt(H)
    b = np.zeros((C,), dtype=np.float32)
    out = kernel(x, wid, W, b)
    print(out.shape, out.dtype)



# revision 2
# speedup vs baseline: 25.1775x; 25.1775x over previous
"""Segment-mean + linear head kernel for TRN2 (8 NeuronCores, data parallel).

Reference (per batch row r):
    pooled[s] = mean over tokens s' with word_id[s']==word_id[s] of x[s'],
    logits = pooled @ W.T + b.

The mean commutes with the linear head, so per row:
    y = x @ W.T              [S, C]   (the only op touching the big tensor)
    out = M @ y + b          [S, C]
where M[s', s] = [word_id[s']==word_id[s]] / cnt(word_id[s]) is the
averaging operator. word_ids are sorted per row, so segments are contiguous
runs and M is block-tridiagonal in 128-token tiles (a run rarely spans >2
tiles; the host computes the exact block list from the data, unioned across
cores so the SPMD program is identical). M blocks are built on the host and
shipped as bf16; the device does projection, segment-mean (as matmuls
against M blocks) and bias.

x is loaded transposed (h on partitions) via the xbar DMA-transpose, so the
tensor engine computes y^T = W @ x^T directly with zero on-chip transposes
of the big tensor. y^T is flipped back to token-major via 16 PE transposes
per row (tiny: [16,128] each).
"""

import sys
from contextlib import ExitStack

import numpy as np

for _p in ("/opt/trn_rl_repo",):
    if _p not in sys.path:
        sys.path.insert(0, _p)

import concourse.bass as bass
import concourse.bacc as bacc
import concourse.tile as tile
from concourse import mybir
from concourse.bass_utils import run_bass_kernel_spmd

B, S, H, C = 16, 2048, 1024, 15
NCORES = 8
RPC = B // NCORES          # rows per core
T = S // 128               # 128-token tiles per row
NK = H // 128              # 128-wide h chunks
CP = 16                    # channels padded

F32 = mybir.dt.float32
BF16 = mybir.dt.bfloat16


def _schedule(word_ids):
    """Per-token inverse segment counts and the (t, t') M-block list.

    Returns (invc [B,S] f32, rid [B,S] int64, blk_list [T][sorted t'] shared
    across rows/cores)."""
    wid = np.asarray(word_ids)
    d = np.diff(wid, axis=1) != 0
    rid = np.concatenate([np.zeros((B, 1), np.int64), np.cumsum(d, axis=1)], axis=1)
    invc = np.empty((B, S), np.float32)
    for r in range(B):
        cnt = np.bincount(rid[r])
        invc[r] = 1.0 / cnt[rid[r]]
    rmin = rid[:, ::128][:, :T]          # rid at tile starts
    rmax = rid[:, 127::128][:, :T]       # rid at tile ends
    # need[t_src, t_dst]: tiles share a run in ANY row
    lo = np.maximum(rmin[:, :, None], rmin[:, None, :])
    hi = np.minimum(rmax[:, :, None], rmax[:, None, :])
    need = (lo <= hi).any(axis=0)        # [T, T] symmetric
    blk_list = [sorted(np.nonzero(need[:, t])[0].tolist()) for t in range(T)]
    return invc, rid, blk_list


def _build(blk_list):
    NB = sum(len(bl) for bl in blk_list)
    nc = bacc.Bacc("TRN2", target_bir_lowering=False, debug=False)
    x_d = nc.declare_dram_parameter("x", [RPC, S, H], BF16, isOutput=False)
    m_d = nc.declare_dram_parameter("m", [RPC, NB, 128, 128], BF16, isOutput=False)
    wt_d = nc.declare_dram_parameter("wt", [NK, 128, CP], BF16, isOutput=False)
    bb_d = nc.declare_dram_parameter("bb", [128, 4 * CP], F32, isOutput=False)
    id_d = nc.declare_dram_parameter("ident", [128, 128], BF16, isOutput=False)
    out_d = nc.declare_dram_parameter("out", [RPC, 128, T * CP], F32, isOutput=True)

    with tile.TileContext(nc) as tc, ExitStack() as ctx:
        consts = ctx.enter_context(tc.tile_pool(name="consts", bufs=1))
        xtp = ctx.enter_context(tc.tile_pool(name="xtp", bufs=2))
        mp = ctx.enter_context(tc.tile_pool(name="mp", bufs=2))
        ysb = ctx.enter_context(tc.tile_pool(name="ysb", bufs=2))
        y1p = ctx.enter_context(tc.tile_pool(name="y1p", bufs=2))
        orp = ctx.enter_context(tc.tile_pool(name="orp", bufs=2))
        yps = ctx.enter_context(tc.tile_pool(name="yps", bufs=2, space="PSUM"))
        tps = ctx.enter_context(tc.tile_pool(name="tps", bufs=2, space="PSUM"))
        ops = ctx.enter_context(tc.tile_pool(name="ops", bufs=2, space="PSUM"))

        wt_sb = consts.tile([128, NK, CP], BF16, tag="wt")
        nc.sync.dma_start(wt_sb[:], wt_d.rearrange("k h c -> h k c"))
        bb_sb = consts.tile([128, 4 * CP], F32, tag="bb")
        nc.sync.dma_start(bb_sb[:], bb_d[:])
        id_sb = consts.tile([128, 128], BF16, tag="ident")
        nc.sync.dma_start(id_sb[:], id_d[:])

        for r in range(RPC):
            # x^T into SBUF, h on partitions: [128, k, S]
            xt = xtp.tile([128, NK, S], BF16, tag="xt")
            for k in range(NK):
                nc.sync.dma_start(
                    xt[:, k, :], x_d[r][:, 128 * k : 128 * k + 128], transpose=True
                )
            m_sb = mp.tile([128, NB, 128], BF16, tag="m")
            nc.sync.dma_start(m_sb[:], m_d[r].rearrange("nb i j -> i nb j"))

            # y^T = W @ x^T : [CP, S] in PSUM, copy (cast bf16) to SBUF
            y_sb = ysb.tile([CP, S], BF16, tag="y")
            for g in range(S // 512):
                yp = yps.tile([CP, 512], F32, tag="yp")
                for k in range(NK):
                    nc.tensor.matmul(
                        yp[:],
                        wt_sb[:, k, :],
                        xt[:, k, 512 * g : 512 * g + 512],
                        start=(k == 0),
                        stop=(k == NK - 1),
                    )
                nc.vector.tensor_copy(y_sb[:, 512 * g : 512 * g + 512], yp[:])

            # y1[t]: [128 tok, CP] via PE transposes, 4 tiles per PSUM buf
            y1 = y1p.tile([128, T // 4, 4 * CP], BF16, tag="y1")
            for q in range(T // 4):
                tp = tps.tile([128, 4 * CP], BF16, tag="tp")
                for i in range(4):
                    t = 4 * q + i
                    nc.tensor.transpose(
                        tp[:, CP * i : CP * i + CP],
                        y_sb[:, 128 * t : 128 * t + 128],
                        id_sb[0:CP, 0:CP],
                    )
                nc.vector.tensor_copy(y1[:, q, :], tp[:])

            # out[t] = sum_{t'} M(t',t)^T y1[t'], + bias during PSUM->SBUF
            orow = orp.tile([128, T * CP], F32, tag="orow")
            nb = 0
            for q in range(T // 4):
                op = ops.tile([128, 4 * CP], F32, tag="op")
                for i in range(4):
                    t = 4 * q + i
                    bl = blk_list[t]
                    for idx, tsrc in enumerate(bl):
                        nc.tensor.matmul(
                            op[:, CP * i : CP * i + CP],
                            m_sb[:, nb, :],
                            y1[:, tsrc // 4, CP * (tsrc % 4) : CP * (tsrc % 4) + CP],
                            start=(idx == 0),
                            stop=(idx == len(bl) - 1),
                        )
                        nb += 1
                nc.vector.tensor_add(
                    orow[:, 4 * CP * q : 4 * CP * q + 4 * CP], op[:], bb_sb[:]
                )
            nc.sync.dma_start(out_d[r], orow[:])

    nc.compile()
    return nc


def _prep_host(x, word_ids, W, b):
    import ml_dtypes

    xb = np.ascontiguousarray(np.asarray(x, dtype=np.float32)).astype(
        ml_dtypes.bfloat16
    )
    invc, rid, blk_list = _schedule(word_ids)
    NB = sum(len(bl) for bl in blk_list)
    m_host = np.empty((B, NB, 128, 128), ml_dtypes.bfloat16)
    nb = 0
    for t in range(T):
        jt = slice(128 * t, 128 * t + 128)
        for tsrc in blk_list[t]:
            js = slice(128 * tsrc, 128 * tsrc + 128)
            eq = rid[:, js, None] == rid[:, None, jt]
            m_host[:, nb] = eq * invc[:, js, None]
            nb += 1
    wtk = np.zeros((NK, 128, CP), np.float32)
    wtk[:, :, :C] = np.asarray(W, dtype=np.float32).T.reshape(NK, 128, C)
    wtk = wtk.astype(ml_dtypes.bfloat16)
    bb = np.zeros((128, 4 * CP), np.float32)
    bb[:, :] = np.tile(
        np.concatenate([np.asarray(b, np.float32), np.zeros(CP - C, np.float32)]), 4
    )[None, :]
    ident = np.eye(128, dtype=np.float32).astype(ml_dtypes.bfloat16)
    return xb, m_host, wtk, bb, ident, blk_list


def _run(x, word_ids, W, b, **spmd_kwargs):
    xb, m_host, wtk, bb, ident, blk_list = _prep_host(x, word_ids, W, b)
    nc = _build(blk_list)

    in_maps = []
    for core in range(NCORES):
        r0 = core * RPC
        in_maps.append(
            {
                "x": xb[r0 : r0 + RPC],
                "m": m_host[r0 : r0 + RPC],
                "wt": wtk,
                "bb": bb,
                "ident": ident,
            }
        )
    res = run_bass_kernel_spmd(nc, in_maps, list(range(NCORES)), **spmd_kwargs)
    outs = []
    for core in range(NCORES):
        o = res.results[core]["out"]  # [RPC, 128, T*CP]
        o = (
            o.reshape(RPC, 128, T, CP)[..., :C]
            .transpose(0, 2, 1, 3)
            .reshape(RPC, S, C)
        )
        outs.append(o)
    full = np.ascontiguousarray(np.concatenate(outs, axis=0).astype(np.float32))
    return full, res


def kernel(x, word_ids, W, b):
    return _run(x, word_ids, W, b)[0]


if __name__ == "__main__":
    rng = np.random.default_rng(0)
    x = rng.standard_normal((B, S, H), dtype=np.float32)
    wid = np.sort(rng.integers(0, 800, (B, S)), axis=-1)
    W = rng.standard_normal((C, H), dtype=np.float32) / np.sqrt(H)
    b = np.zeros((C,), dtype=np.float32)
    out = kernel(x, wid, W, b)
    print(out.shape, out.dtype)
